# revision 67
# baseline (speedup 1.0000x reference)
"""Causal multi-head attention (B=2, S=2048, D=1024, H=16, HD=64) on 8 NeuronCores.

Sharding: core c = 4*b + g handles batch b (2-way data parallel) and head
group g (4-way tensor parallel, 4 heads per core). Each core computes its
heads' attention plus the partial output projection; the host sums the 4
partials per batch and adds the bias.

Implementation notes:
  - QKV projections run as 3-term fp8e4m3 DoubleRow matmuls:
    X ~ Xh+Xl, 32W ~ Wh+Wl (hi/lo e4m3 splits, host-prepared), and
    out = Wh.X + Wl.Xh + Wh.Xl accumulates in one PSUM group (all terms
    share the 32x scale, folded into the exp scale / V ones-column).
  - Q,K,V are stored fp16 at 32x scale; scores are 1024x scaled.
  - softmax exp splits between the scalar engine (exact, scale folded) and
    the vector engine (Schraudolph int16 bit-trick into fp16 bits, ~3% max
    err); causal masking multiplies diagonal tiles by a triangular fp16
    mask on gpsimd.
  - the P@V matmul uses V augmented with a 32.0 column so the same matmul
    yields the softmax denominator at the true scale.
  - normalization per head: DVE copies the l row, DVE reciprocal, gpsimd
    partition broadcast, DVE multiply into fp16 ctxn.
  - output projection partials drain to fp16 via ACT/DVE (balanced) and
    DMA to DRAM; the host sums the fp16 partials in f32 and adds bias.
  - engine choice for exp tiles alternates ACT/DVE; QK drains go to ACT,
    the normalization chain lives on DVE, masks/broadcasts on gpsimd;
    output-projection units are interleaved late in the schedule as PE
    fillers, where softmax exp pressure would otherwise stall the PE.
"""

import numpy as np
import ml_dtypes

import concourse.mybir as mybir
from concourse import bacc
from concourse.tile import TileContext
from concourse.bass_utils import run_bass_kernel_spmd
from concourse.masks import make_upper_triangular

F32 = mybir.dt.float32
F16 = mybir.dt.float16
I16 = mybir.dt.int16
E4 = mybir.dt.float8e4
Exp = mybir.ActivationFunctionType.Exp
Copy = mybir.ActivationFunctionType.Copy
Alu = mybir.AluOpType
DR = mybir.MatmulPerfMode.DoubleRow

B, S, D, H, HD = 2, 2048, 1024, 16, 64
GH = 4            # heads per core
GD = GH * HD      # 256 features per core
N_CORES = 8

# exp(s_raw/8) with s_raw = s/1024 (Q,K carry 32x scale each)
EXP_SCALE = 0.125 / 1024.0
SCH_A = float(1024.0 * np.log2(np.e) * EXP_SCALE)
SCH_B = float(15 * 1024 - 0.0439 * 1024)
E4NP = ml_dtypes.float8_e4m3


def _build():
    nc = bacc.Bacc("TRN2", target_bir_lowering=False, name="mha_tp2")
    # X^T hi/lo splits, rows pre-permuted to [p, (ddt, sub, s)] on the host
    xh_d = nc.dram_tensor("x8h", [128, 8 * S], E4, kind="ExternalInput")
    xl_d = nc.dram_tensor("x8l", [128, 8 * S], E4, kind="ExternalInput")
    w_d = {}
    for wn in ("q", "k", "v"):
        for hl in ("h", "l"):
            w_d[wn + hl] = nc.dram_tensor(
                f"w{wn}{hl}", [128, 8 * GD], E4, kind="ExternalInput"
            )
    wo_d = nc.dram_tensor("wo", [128, 2 * D], F16, kind="ExternalInput")
    out_d = nc.dram_tensor("out", [S, D], F16, kind="ExternalOutput")

    with TileContext(nc) as tc:
        with (
            tc.tile_pool(name="per", bufs=1) as per,
            tc.tile_pool(name="pt", bufs=32) as ptp,
            tc.tile_pool(name="nrm", bufs=4) as nrm,
            tc.tile_pool(name="obp", bufs=6) as obp,
            tc.tile_pool(name="ps_a", bufs=2, space="PSUM") as ps_a,
            tc.tile_pool(name="ps_c", bufs=2, space="PSUM") as ps_c,
            tc.tile_pool(name="ps_q", bufs=2, space="PSUM") as ps_q,
        ):
            x8h = per.tile([128, 4, 2, S], E4)
            x8l = per.tile([128, 4, 2, S], E4)
            w8 = {
                k: per.tile([128, 4, 2, GD], E4, name=f"w8{k}") for k in w_d
            }
            wo = per.tile([128, 2, D], F16)
            qt = per.tile([128, 2, S], F16)       # 32*Q^T, pair-major
            kt = per.tile([128, 2, S], F16)
            vaug = per.tile([128, 16, GH * (HD + 1)], F16)  # 32*V + 32.0 col
            ctxn = per.tile([128, 2, S], F16)     # normalized ctx^T (true scale)
            tri2 = per.tile([128, 256], F16)      # [tri | tri]
            warm = per.tile([128, 512], F16)

            # ---- input DMAs (SP), consumption order ----
            def dma_x(t, d, sc):
                nc.sync.dma_start(
                    t[:, :, :, 512 * sc:512 * sc + 512],
                    d[:, :].rearrange("p (a s n) -> p a s n", a=4, s=2)[
                        :, :, :, 512 * sc:512 * sc + 512
                    ],
                )

            def dma_w(name):
                nc.sync.dma_start(
                    w8[name][:, :, :, :],
                    w_d[name][:, :].rearrange("p (a s n) -> p a s n", a=4, s=2),
                )

            dma_x(x8h, xh_d, 0)
            dma_w("qh")
            dma_w("ql")
            dma_x(x8l, xl_d, 0)
            dma_w("kh")
            dma_w("kl")
            dma_w("vh")
            dma_w("vl")
            for sc in range(1, 4):
                dma_x(x8h, xh_d, sc)
                dma_x(x8l, xl_d, sc)
            nc.sync.dma_start(
                wo[:, :, :], wo_d[:, :].rearrange("p (t d) -> p t d", t=2)
            )

            # ---- constants + PE warmup ----
            nc.gpsimd.memset(warm[:, :], 1.0)
            make_upper_triangular(nc, tri2[:, 0:128], val=1.0, diag=True)
            make_upper_triangular(nc, tri2[:, 128:256], val=1.0, diag=True)
            vones = vaug.rearrange("p st (h c) -> p st h c", c=HD + 1)
            nc.gpsimd.memset(vones[:, :, :, HD:HD + 1], 32.0)

            wps = ps_a.tile([128, 1024], F32, tag="blk")
            N_WARM = 1
            for i in range(N_WARM):
                nc.tensor.matmul(
                    wps[:, 0:512], warm[:, 0:128], warm[:, 0:512],
                    start=(i == 0), stop=(i == N_WARM - 1),
                )

            # ---- QKV projections: 3-term fp8 DoubleRow ----
            def emit_qk(wn, dst, sc):
                for dp in range(2):
                    qp = ps_q.tile([128, 512], F32, tag="po")
                    n = 0
                    for wt, xt_ in ((w8[wn + "h"], x8h), (w8[wn + "l"], x8h),
                                    (w8[wn + "h"], x8l)):
                        for ddt in range(4):
                            nc.tensor.matmul(
                                qp[:, :],
                                wt[:, ddt, :, 128 * dp:128 * dp + 128],
                                xt_[:, ddt, :, 512 * sc:512 * sc + 512],
                                start=(n == 0), stop=(n == 11),
                                perf_mode=DR,
                            )
                            n += 1
                    load["act"] += 612
                    nc.scalar.copy(dst[:, dp, 512 * sc:512 * sc + 512], qp[:, :])

            def emit_v(sc):
                for tp in range(2):
                    st0 = 4 * sc + 2 * tp
                    vp = ps_q.tile([128, 512], F32, tag="po")
                    for k in range(2):
                        n = 0
                        for wt, xt_ in ((w8["vh"], x8h), (w8["vl"], x8h),
                                        (w8["vh"], x8l)):
                            for ddt in range(4):
                                nc.tensor.matmul(
                                    vp[:, 256 * k:256 * k + 256],
                                    xt_[:, ddt, :,
                                        128 * (st0 + k):128 * (st0 + k) + 128],
                                    wt[:, ddt, :, :],
                                    start=(n == 0), stop=(n == 11),
                                    perf_mode=DR,
                                )
                                n += 1
                    load["dve"] += 658
                    v_dst = vaug[:, st0:st0 + 2, :].rearrange(
                        "p st (h c) -> p st h c", c=HD + 1)
                    nc.vector.tensor_copy(
                        v_dst[:, :, :, 0:HD],
                        vp[:, :].rearrange("p (st h c) -> p st h c", st=2, c=HD),
                    )

            # ---- attention ----
            load = {"act": 0.0, "dve": 0.0}

            def pick_eng(act_cost, dve_cost, force=None):
                eng = force or ("act" if load["act"] + act_cost
                                <= load["dve"] + dve_cost else "dve")
                load[eng] += act_cost if eng == "act" else dve_cost
                return eng

            exp_rr = [0]

            def emit_exp(sp, pt_t, w, force=None):
                if force is None:
                    force = "act" if exp_rr[0] % 2 == 0 else "dve"
                    exp_rr[0] += 1
                eng = pick_eng(w * 0.8333 + 185, w * 1.0417 + 125, force)
                if eng == "act":
                    nc.scalar.activation(pt_t[:, :w], sp[:, :w], Exp,
                                         scale=EXP_SCALE)
                else:
                    nc.vector.tensor_scalar(
                        pt_t.bitcast(I16)[:, :w], sp[:, :w], SCH_A, SCH_B,
                        op0=Alu.mult, op1=Alu.add,
                    )

            def emit_scores(h, qc, fillers):
                """Score blocks + exp for head h chunk qc. Returns pt list."""
                i, qo = h // 2, 64 * (h % 2)
                out = []
                # diagonal strips: pack0 [896] (j0 w512 / j1 w384),
                #                  pack1 [384] (j3 w128 / j2 w256)
                packs = (
                    (896, ((0, 0, 512), (1, 512, 384)),
                     ((0, 128), (512, 128))),
                    (384, ((3, 0, 128), (2, 128, 256)),
                     ((0, 256),)),
                )
                for width, parts, masks in packs:
                    sp = ps_a.tile([128, 1024], F32, tag="blk")
                    for j, o, w in parts:
                        k_t = 4 * qc + j
                        nc.tensor.matmul(
                            sp[:, o:o + w],
                            kt[qo:qo + 64, i, 128 * k_t:128 * k_t + 128],
                            qt[qo:qo + 64, i,
                               512 * qc + 128 * j:512 * qc + 128 * j + w],
                            start=True, stop=True,
                        )
                    pt_t = ptp.tile([128, 1024], F16, tag="pt")
                    emit_exp(sp, pt_t, width)
                    for mo, mw in masks:
                        nc.gpsimd.tensor_mul(
                            pt_t[:, mo:mo + mw], pt_t[:, mo:mo + mw],
                            tri2[:, 0:mw],
                        )
                    out.append((pt_t, parts, True))
                # full 1024-wide blocks (2 k-tiles each)
                for blk in range(2 * qc):
                    sp = ps_a.tile([128, 1024], F32, tag="blk")
                    for j2 in range(2):
                        k_t = 2 * blk + j2
                        nc.tensor.matmul(
                            sp[:, 512 * j2:512 * j2 + 512],
                            kt[qo:qo + 64, i, 128 * k_t:128 * k_t + 128],
                            qt[qo:qo + 64, i, 512 * qc:512 * qc + 512],
                            start=True, stop=True,
                        )
                    pt_t = ptp.tile([128, 1024], F16, tag="pt")
                    emit_exp(sp, pt_t, 1024)
                    out.append((pt_t, ((2 * blk, 0, 512), (2 * blk + 1, 512, 512)),
                                False))
                    if fillers:
                        fillers.pop(0)()
                if fillers:
                    fillers.pop(0)()
                return out

            def emit_pv(h, qc, pts):
                i = h // 2
                ctx = ps_c.tile([65, 512], F32, tag="ctx")
                mms = []
                for pt_t, parts, is_diag in pts:
                    for j, o, w in parts:
                        if is_diag:
                            k_t, co = 4 * qc + j, 128 * j
                        else:
                            k_t, co = j, 0
                        mms.append((pt_t, k_t, o, w, co))
                for n, (pt_t, k_t, o, w, co) in enumerate(mms):
                    nc.tensor.matmul(
                        ctx[:, co:co + w],
                        vaug[:, k_t, 65 * h:65 * h + 65],
                        pt_t[:, o:o + w],
                        start=(n == 0), stop=(n == len(mms) - 1),
                    )
                return ctx

            def emit_norm_head(h, qc, ctx):
                i, row = h // 2, 64 * (h % 2)
                load["dve"] += 2600
                l1 = nrm.tile([1, 512], F32, tag="l1")
                nc.vector.tensor_copy(l1[:, :], ctx[64:65, :])
                r1 = nrm.tile([1, 512], F32, tag="r1")
                nc.vector.reciprocal_approx_fast(r1[:, :], l1[:1, :])
                rb = nrm.tile([64, 512], F32, tag="rb")
                nc.gpsimd.partition_broadcast(rb[:, :], r1[:1, :], channels=64)
                nc.vector.tensor_mul(
                    ctxn[row:row + 64, i, 512 * qc:512 * qc + 512],
                    ctx[0:64, :], rb[:, :],
                )

            def emit_attn_pair(i, qc, fillers=None):
                fillers = fillers or []
                hA, hB = 2 * i, 2 * i + 1
                ptsA = emit_scores(hA, qc, fillers)
                ptsB = emit_scores(hB, qc, fillers)
                ctxA = emit_pv(hA, qc, ptsA)
                emit_norm_head(hA, qc, ctxA)
                ctxB = emit_pv(hB, qc, ptsB)
                emit_norm_head(hB, qc, ctxB)
                while fillers:
                    fillers.pop(0)()

            def op_unit(st, oc, force_eng=None):
                def emit():
                    po = ps_q.tile([128, 512], F32, tag="po")
                    for dp in range(2):
                        nc.tensor.matmul(
                            po[:, :],
                            ctxn[:, dp, 128 * st:128 * st + 128],
                            wo[:, dp, 512 * oc:512 * oc + 512],
                            start=(dp == 0), stop=(dp == 1),
                        )
                    ob = obp.tile([128, 512], F16, tag="ob")
                    if pick_eng(612, 658, force=force_eng) == "act":
                        nc.scalar.copy(ob[:, :], po[:, :])
                    else:
                        nc.vector.tensor_copy(ob[:, :], po[:, :])
                    nc.sync.dma_start(
                        out_d[128 * st:128 * st + 128, 512 * oc:512 * oc + 512],
                        ob[:, :],
                    )
                return emit

            def emit_outproj(qc, force_eng=None):
                for st in range(4 * qc, 4 * qc + 4):
                    for oc in range(2):
                        op_unit(st, oc, force_eng)()

            # ---- schedule ----
            emit_qk("q", qt, 0)
            emit_qk("k", kt, 0)
            emit_v(0)
            emit_attn_pair(0, 0)
            emit_attn_pair(1, 0)
            emit_qk("q", qt, 1)
            emit_qk("k", kt, 1)
            emit_v(1)
            emit_attn_pair(0, 1)
            emit_qk("q", qt, 2)
            emit_qk("k", kt, 2)
            emit_v(2)
            emit_attn_pair(0, 2)
            emit_qk("q", qt, 3)
            emit_qk("k", kt, 3)
            emit_v(3)
            op0 = [op_unit(st, oc) for st in range(0, 4) for oc in range(2)]
            emit_attn_pair(0, 3, fillers=op0)
            op2 = [op_unit(st, oc) for st in range(8, 12) for oc in range(2)]
            emit_attn_pair(1, 2)
            emit_attn_pair(1, 3, fillers=op2)
            op3 = [op_unit(st, oc) for st in range(12, 16) for oc in range(2)]
            emit_attn_pair(1, 1, fillers=op3)
            emit_outproj(1)
    nc.compile()
    return nc


_NC = None


def _get_nc():
    global _NC
    if _NC is None:
        _NC = _build()
    return _NC


def _split8(a):
    hi = a.astype(E4NP)
    lo = (a - hi.astype(np.float32)).astype(E4NP)
    return hi, lo


def _dev_layout(a):
    """[1024, N] -> [128, 8*N] with row p holding (ddt, sub) blocks."""
    n = a.shape[1]
    return np.ascontiguousarray(
        a.reshape(4, 2, 128, n).transpose(2, 0, 1, 3).reshape(128, 8 * n)
    )


def build_in_maps(inputs):
    x = np.asarray(inputs["inputs"], dtype=np.float32)
    wq = np.asarray(inputs["Wq"], dtype=np.float32)
    wk = np.asarray(inputs["Wk"], dtype=np.float32)
    wv = np.asarray(inputs["Wv"], dtype=np.float32)
    wo = np.asarray(inputs["Wo"], dtype=np.float32)

    xparts = []
    for b in range(B):
        xh, xl = _split8(np.ascontiguousarray(x[b].T))
        xparts.append((_dev_layout(xh.view(np.uint8)).view(E4NP),
                       _dev_layout(xl.view(np.uint8)).view(E4NP)))

    in_maps = []
    for c in range(N_CORES):
        b, g = c // 4, c % 4
        sl = slice(GD * g, GD * g + GD)
        m = {"x8h": xparts[b][0], "x8l": xparts[b][1]}
        for wn, wmat in (("q", wq), ("k", wk), ("v", wv)):
            wh, wl = _split8(np.ascontiguousarray(wmat[sl, :].T) * 32.0)
            m[f"w{wn}h"] = _dev_layout(wh.view(np.uint8)).view(E4NP)
            m[f"w{wn}l"] = _dev_layout(wl.view(np.uint8)).view(E4NP)
        wog = np.ascontiguousarray(wo[:, sl].T).astype(np.float16)
        m["wo"] = np.ascontiguousarray(
            wog.reshape(2, 128, D).transpose(1, 0, 2).reshape(128, 2 * D)
        )
        in_maps.append(m)
    return in_maps


def kernel(**inputs):
    bo = np.asarray(inputs["bo"], dtype=np.float32)
    in_maps = build_in_maps(inputs)
    nc = _get_nc()
    res = run_bass_kernel_spmd(nc, in_maps, core_ids=list(range(N_CORES)))
    out = np.empty((B, S, D), np.float32)
    for b in range(B):
        acc = res.results[4 * b + 0]["out"].astype(np.float32)
        for g in range(1, 4):
            acc = acc + res.results[4 * b + g]["out"]
        out[b] = acc + bo
    return out
